# revision 25
# baseline (speedup 1.0000x reference)
"""Trainium2 Bass kernel for nn_AttentionBlock (B=4, H=W=64, C=512).

Strategy (8 cores, no collectives):
  - 2 cores per batch image; each core handles 2048 of the 4096 queries.
  - Key/token order is permuted per core so that each core's OWN query rows
    are tokens 0..2047 of its private x copy (softmax is invariant to key
    permutation as long as K and V use the same order).
  - All GEMMs run in fp8e4 with MatmulPerfMode.DoubleRow: lhsT/rhs carry
    [128, 2, *] channel- or key-chunk pairs so each matmul contracts 256
    elements at 1 column/cycle (the 157 TF/s fp8 peak).
  - Per core: LayerNorm (bn_stats on DVE; rstd via one optimized
    fast-inverse-sqrt Newton step), transpose hn to channel-major hfT
    (fp8 PE transpose), Q^T/K^T (channel-major) projections in fp8:
        S^T[k,q] = K^T.T @ Q^T     (PSUM fp32)
        P^T = exp(S^T/sqrt(C)-2.9) (ACT, scale+shift folded into the table)
        O^T[c,q] += V.T-pair @ P^T (PSUM planes 0-3, no output transpose)
        sums[q]  += ones.T @ P^T   (separate 1-bank PSUM tile)
        y = (O^T fp8) proj via 4*Wp back to [q,c]; y *= 1/(4*sums);
        out = y + x + const-biases
  - KEY SCHEDULING FACT: writes to one SBUF tile from different engines
    serialize in emission order (the tile tracker is cross-engine
    order-preserving per tile). All per-token-tile arrays (hn, hfT, K^T,
    Q^T) are therefore allocated as PER-TILE tiles with a SINGLE writer
    engine each, alternating engines across tiles, so ACT/DVE/GpSimd
    drain independent tiles concurrently. The O^T fp8 drain is likewise
    split into two tiles (planes 01 -> ACT, planes 23 -> DVE).
  - Tile boundaries: FIVE score pairs are prebuilt so the PE has a ~4.3us
    runway while the previous tile's PSUM is drained; the previous y
    projection/epilogue is emitted right after the first PV.
  - LN gamma/beta are folded into the QKV weights/biases on the host;
    bv/bp biases are folded into the residual input xr on the host; the
    softmax 1/sqrt(C) scale is applied by the ACT exp instruction.
"""

import os
import sys

import numpy as np
import ml_dtypes

try:
    import concourse.bass as bass
except ImportError:  # pragma: no cover - fresh-dir fallback
    for _p in ("/opt/trn_rl_repo", "/root/.axon_site/_ro/trn_rl_repo"):
        if os.path.isdir(_p) and _p not in sys.path:
            sys.path.insert(0, _p)
    import concourse.bass as bass

import concourse.bacc as bacc
import concourse.tile as tile
from concourse import mybir
from concourse.bass_utils import run_bass_kernel_spmd

F32 = mybir.dt.float32
BF16 = mybir.dt.bfloat16
F8 = mybir.dt.float8e4
AF = mybir.ActivationFunctionType
ALU = mybir.AluOpType
DR = mybir.MatmulPerfMode.DoubleRow
NPF8 = ml_dtypes.float8_e4m3

B, Hh, Ww, C = 4, 64, 64, 512
N_TOK = Hh * Ww          # 4096 tokens per image
NCORES = 8
NQ = N_TOK * B // NCORES  # 2048 queries per core
LN_EPS = 1e-3
CI = C // 128             # 4 channel chunks
SSCALE = 1.0 / float(np.sqrt(np.float32(C)))  # softmax scale, applied in exp
# exp(S*scale + ESHIFT): keeps P <= ~30 and O^T <= ~150 (fp8e4 max 240),
# so the O^T PSUM->SBUF copy is a pure cast. The extra ln(4) is undone by
# 4*Wp and the 4x srow scale (normalization is scale-invariant).
ESHIFT = -(1.5 + float(np.log(4.0)))
WPSCALE = 4.0

LAST_EXEC_NS = None
LAST_RESULT = None


def build_program(n_tok=N_TOK, nq=NQ):
    """Build the per-core Bass program (identical across cores)."""
    assert n_tok % 1024 == 0 and nq % 512 == 0
    nt_tiles = n_tok // 512   # n-tiles for K/V over all tokens
    qt_tiles = nq // 512      # q-tiles for this core's queries
    kc_n = n_tok // 128       # key chunks
    kp_n = kc_n // 2          # key chunk pairs

    nc = bacc.Bacc()
    if os.environ.get("BASS_CACHE_BUST"):
        nc.dram_tensor(f"cachebust_{os.environ['BASS_CACHE_BUST']}", [1, 1], F32)
    x_d = nc.dram_tensor("x", [n_tok, C], F32, kind="ExternalInput")
    xr_d = nc.dram_tensor("xr", [nq, C], F32, kind="ExternalInput")
    wq_d = nc.dram_tensor("wq", [C, C], F8, kind="ExternalInput")
    wk_d = nc.dram_tensor("wk", [C, C], F8, kind="ExternalInput")
    wvp_d = nc.dram_tensor("wvp", [C, C], F8, kind="ExternalInput")
    bq_d = nc.dram_tensor("bq", [128, CI], F32, kind="ExternalInput")
    bk_d = nc.dram_tensor("bk", [128, CI], F32, kind="ExternalInput")
    id_d = nc.dram_tensor("ident", [128, 128], BF16, kind="ExternalInput")
    id8_d = nc.dram_tensor("ident8", [128, 128], F8, kind="ExternalInput")
    on_d = nc.dram_tensor("ones", [128, 2, 128], F8, kind="ExternalInput")
    y_d = nc.dram_tensor("y", [nq, C], F32, kind="ExternalOutput")

    # token index mapping: tok = tile*512 + k*128 + p  (p = partition)
    x_re = x_d[:].rearrange("(t k p) c -> p t k c", p=128, k=4)
    xr_re = xr_d[:].rearrange("(t k p) c -> p t k c", p=128, k=4)
    y_re = y_d[:].rearrange("(t k p) c -> p t k c", p=128, k=4)

    from contextlib import ExitStack

    with ExitStack() as ctx:
        tc = ctx.enter_context(tile.TileContext(nc))
        consts = ctx.enter_context(tc.tile_pool(name="consts", bufs=1))
        big = ctx.enter_context(tc.tile_pool(name="big", bufs=1))
        work = ctx.enter_context(tc.tile_pool(name="work", bufs=3))
        stat = ctx.enter_context(tc.tile_pool(name="stat", bufs=4))
        ptp = ctx.enter_context(tc.tile_pool(name="ptp", bufs=6))
        ptc = ctx.enter_context(tc.tile_pool(name="ptc", bufs=kp_n))
        epi = ctx.enter_context(tc.tile_pool(name="epi", bufs=3))
        psS = ctx.enter_context(tc.tile_pool(name="psS", bufs=3, space="PSUM"))

        # ---- pipeline head: first x chunk, transpose identity, rest of
        # ---- x tiles 0/1 (chunk-grained so stats start early), weights.
        x_t0 = work.tile([128, 4, C], F32, tag="x", bufs=6)
        nc.sync.dma_start(out=x_t0[:, 0, :], in_=x_re[:, 0, 0, :])
        ident8 = consts.tile([128, 128], F8)
        nc.sync.dma_start(out=ident8, in_=id8_d[:])
        for k in range(1, 4):
            nc.sync.dma_start(out=x_t0[:, k, :], in_=x_re[:, 0, k, :])
        x_t1 = work.tile([128, 4, C], F32, tag="x", bufs=6, name="x_1")
        nc.sync.dma_start(out=x_t1[:, 0:2, :], in_=x_re[:, 1, 0:2, :])
        nc.sync.dma_start(out=x_t1[:, 2:4, :], in_=x_re[:, 1, 2:4, :])
        wk_sb = consts.tile([128, CI, C], F8)
        nc.sync.dma_start(out=wk_sb, in_=wk_d[:].rearrange("(ci p) co -> p ci co", p=128))
        wq_sb = consts.tile([128, CI, C], F8)
        nc.sync.dma_start(out=wq_sb, in_=wq_d[:].rearrange("(ci p) co -> p ci co", p=128))
        bk_sb = consts.tile([128, CI], F32)
        nc.sync.dma_start(out=bk_sb, in_=bk_d[:])
        bq_sb = consts.tile([128, CI], F32)
        nc.sync.dma_start(out=bq_sb, in_=bq_d[:])

        # stage-C constants: tiles now, DMAs issued after the first x
        # prefetches (they are not needed for tens of microseconds)
        wvp_sb = consts.tile([128, CI, C], F8)
        ones8 = consts.tile([128, 2, 128], F8)
        ident = consts.tile([128, 128], BF16)
        shf_sb = consts.tile([128, 1], F32)
        nc.vector.memset(shf_sb, ESHIFT)

        # ---- persistent activations: PER-TILE tiles, single writer each
        hfTs = [big.tile([128, CI, 512], F8, tag=f"hfT{t}", name=f"hfT{t}")
                for t in range(nt_tiles)]
        kTs = [big.tile([128, CI, 512], F8, tag=f"kT{t}", name=f"kT{t}")
               for t in range(nt_tiles)]
        hNs = [big.tile([128, 4, C], F8, tag=f"hN{t}", name=f"hN{t}")
               for t in range(nt_tiles)]
        qTs = [big.tile([128, CI, 512], F8, tag=f"qT{t}", name=f"qT{t}")
               for t in range(qt_tiles)]

        # scores + exp for one key chunk; pipelined ahead of PV use
        def st_exp(qt, kc, pt2, plane):
            kt_t, kcl = kTs[kc // 4], kc % 4
            s_ps = psS.tile([128, 512], F32, tag="st",
                            name=f"s_ps_{qt}_{kc}")
            for ip in range(CI // 2):
                nc.tensor.matmul(
                    s_ps,
                    lhsT=kt_t[:, 2 * ip:2 * ip + 2,
                              kcl * 128:(kcl + 1) * 128],
                    rhs=qTs[qt][:, 2 * ip:2 * ip + 2, :],
                    perf_mode=DR,
                    start=(ip == 0), stop=(ip == CI // 2 - 1))
            nc.scalar.activation(out=pt2[:, plane, :], in_=s_ps,
                                 func=AF.Exp, scale=SSCALE,
                                 bias=shf_sb)

        def make_pair(qt, p, pool):
            pt2 = pool.tile([128, 2, 512], F8, tag="pt",
                            name=f"pt_{qt}_{p}")
            st_exp(qt, 2 * p, pt2, 0)
            st_exp(qt, 2 * p + 1, pt2, 1)
            return pt2

        # ========= Stage A+B: LN, transpose, projections; the scores+exp
        # ========= for query tile 0 are interleaved as kT chunks land.
        pt0_cache = []
        xtiles = {0: x_t0, 1: x_t1}

        def fetch_x(t):
            if t not in xtiles and t < nt_tiles:
                xt = work.tile([128, 4, C], F32, tag="x", bufs=6,
                               name=f"x_{t}")
                nc.sync.dma_start(out=xt, in_=x_re[:, t, :, :])
                xtiles[t] = xt

        def emit_stats_n(chunks, label):
            """bn_stats (DVE) + one optimized fast-inverse-sqrt step.

            Kadlec's RcpSqrt1 (magic 0x5F1FFFF9, y*(1.68191409 -
            0.703952253*v*y^2)): max rel err 6.5e-4, well inside the fp8
            noise floor, and only 7 serial DVE micro-ops per batch.
            """
            m = len(chunks)
            mv8 = stat.tile([128, m, 2], F32, tag="mv", name=f"mv_{label}")
            for i, (xt, k) in enumerate(chunks):
                stats = stat.tile([128, 6], F32, tag="bnst")
                nc.vector.bn_stats(out=stats, in_=xt[:, k, :])
                nc.vector.bn_aggr(out=mv8[:, i, :], in_=stats)
            I32 = mybir.dt.int32
            veps = stat.tile([128, m], F32, tag="veps")
            nc.vector.tensor_scalar_add(out=veps, in0=mv8[:, :, 1],
                                        scalar1=LN_EPS)
            yb = stat.tile([128, m], I32, tag="yb")
            nc.vector.tensor_scalar(out=yb, in0=veps[:].bitcast(I32),
                                    scalar1=1, scalar2=None,
                                    op0=ALU.logical_shift_right)
            y0b = stat.tile([128, m], I32, tag="y0b")
            nc.vector.tensor_scalar(out=y0b, in0=yb, scalar1=0x5F1FFFF9,
                                    scalar2=-1,
                                    op0=ALU.subtract, op1=ALU.mult)
            t1 = stat.tile([128, m], F32, tag="nt1")
            nc.vector.tensor_tensor(out=t1, in0=y0b[:].bitcast(F32),
                                    in1=y0b[:].bitcast(F32), op=ALU.mult)
            t2 = stat.tile([128, m], F32, tag="nt2")
            nc.vector.tensor_tensor(out=t2, in0=t1, in1=veps, op=ALU.mult)
            t3 = stat.tile([128, m], F32, tag="nt3")
            nc.vector.tensor_scalar(out=t3, in0=t2, scalar1=-0.703952253,
                                    scalar2=1.68191409,
                                    op0=ALU.mult, op1=ALU.add)
            rstd8 = stat.tile([128, m], F32, tag="rstd", name=f"rstd_{label}")
            nc.vector.tensor_tensor(out=rstd8, in0=y0b[:].bitcast(F32),
                                    in1=t3, op=ALU.mult)
            # hn is applied as Identity(x*rstd + (-mu*rstd))
            b8 = stat.tile([128, m], F32, tag="b8", name=f"b8_{label}")
            nc.vector.scalar_tensor_tensor(out=b8, in0=mv8[:, :, 0],
                                           scalar=-1.0, in1=rstd8,
                                           op0=ALU.mult, op1=ALU.mult)
            return rstd8, b8

        ln_aff = {}
        with tc.tile_pool(name="psAB", bufs=5, space="PSUM") as psAB:
            for tp in range(nt_tiles // 2):
                # prefetch x deeply so tile boundaries never wait on HBM
                for ft in range(2 * tp + 2, min(2 * tp + 6, nt_tiles)):
                    fetch_x(ft)
                if tp == 0:
                    # stage-C constants ride behind the first prefetches
                    nc.sync.dma_start(out=wvp_sb, in_=wvp_d[:].rearrange(
                        "(ci p) co -> p ci co", p=128))
                    nc.sync.dma_start(out=ones8, in_=on_d[:])
                    nc.sync.dma_start(out=ident, in_=id_d[:])
                    # chunk 0 alone: its stats depend only on the first
                    # 256KB of x, so the first transpose starts early
                    t0_aff = []
                    for k in range(4):
                        rk, ck = emit_stats_n([(xtiles[0], k)], f"c{k}")
                        t0_aff.append((rk, ck, 0))
                    # tile 1 in two 2-chunk batches (chunk DMAs land in order)
                    r1a, c1a = emit_stats_n([(xtiles[1], 0), (xtiles[1], 1)],
                                            "t1a")
                    r1b, c1b = emit_stats_n([(xtiles[1], 2), (xtiles[1], 3)],
                                            "t1b")
                    ln_aff[0] = [t0_aff,
                                 [(r1a, c1a, 0), (r1a, c1a, 1),
                                  (r1b, c1b, 0), (r1b, c1b, 1)]]
                aff = ln_aff.pop(tp)
                for ti in range(2):
                    t = 2 * tp + ti
                    x_t = xtiles[t]
                    # next pair's stats: one 4-chunk batch per half-tile so
                    # the first tile's rstd lands well before the boundary
                    # (at tp==0 both batches wait until ti==1: tile 2's x
                    # DMA is still behind the startup head, and a blocked
                    # bn_stats would stall the DVE's small lookahead window)
                    if tp + 1 < nt_tiles // 2:
                        if ti == 0 and tp > 0:
                            ra, ca = emit_stats_n(
                                [(xtiles[2 * tp + 2], k) for k in range(4)],
                                f"pa{tp + 1}")
                            ln_aff.setdefault(tp + 1, [None, None])[0] = \
                                [(ra, ca, k) for k in range(4)]
                        elif ti == 1:
                            if tp == 0:
                                ra, ca = emit_stats_n(
                                    [(xtiles[2], k) for k in range(4)], "pa1")
                                ln_aff.setdefault(1, [None, None])[0] = \
                                    [(ra, ca, k) for k in range(4)]
                            rb, cb = emit_stats_n(
                                [(xtiles[2 * tp + 3], k) for k in range(4)],
                                f"pb{tp + 1}")
                            ln_aff[tp + 1][1] = [(rb, cb, k)
                                                 for k in range(4)]
                    # single-writer-per-tile engine assignment:
                    #   LN: ACT for tiles 0-1 (startup latency), then GpSimd
                    #   transpose copies: always DVE
                    #   K drains: always ACT; Q drains: DVE even t, ACT odd t
                    # only tile 0's LN on ACT (shortest cold-start path);
                    # tile 1's would queue behind tile 0's ACT drains
                    ln_eng = nc.scalar if t < 1 else nc.gpsimd
                    for k in range(4):
                        rstd8, b8, idx = aff[ti][k]
                        if ln_eng is nc.scalar:
                            nc.scalar.activation(out=hNs[t][:, k, :],
                                                 in_=x_t[:, k, :],
                                                 func=AF.Identity,
                                                 scale=rstd8[:, idx:idx + 1],
                                                 bias=b8[:, idx:idx + 1])
                        else:
                            nc.gpsimd.tensor_scalar(
                                out=hNs[t][:, k, :], in0=x_t[:, k, :],
                                scalar1=rstd8[:, idx:idx + 1],
                                scalar2=b8[:, idx:idx + 1],
                                op0=ALU.mult, op1=ALU.add)
                        # fp8 transpose must write PSUM at element step 2
                        tr_ps = psAB.tile([128, CI, 128, 2], F8, tag="ps")
                        for j in range(CI):
                            nc.tensor.transpose(
                                tr_ps[:, j, :, 0],
                                hNs[t][:, k, j * 128:(j + 1) * 128],
                                ident8)
                        nc.vector.tensor_copy(
                            out=hfTs[t][:, :, k * 128:(k + 1) * 128],
                            in_=tr_ps[:, :, :, 0])

                    # K^T columns for this tile (all drains on ACT: kTs[t]
                    # has a single writer engine)
                    for j in range(CI):
                        k_ps = psAB.tile([128, 512], F32, tag="ps")
                        for ip in range(CI // 2):
                            nc.tensor.matmul(
                                k_ps,
                                lhsT=wk_sb[:, 2 * ip:2 * ip + 2,
                                           j * 128:(j + 1) * 128],
                                rhs=hfTs[t][:, 2 * ip:2 * ip + 2, :],
                                perf_mode=DR,
                                start=(ip == 0), stop=(ip == CI // 2 - 1))
                        nc.scalar.activation(
                            out=kTs[t][:, j, :],
                            in_=k_ps, func=AF.Identity,
                            bias=bk_sb[:, j:j + 1])

                    # Q^T columns (only for this core's query range);
                    # drains on DVE for even t, ACT for odd t
                    if t < qt_tiles:
                        for j in range(CI):
                            q_ps = psAB.tile([128, 512], F32, tag="ps")
                            for ip in range(CI // 2):
                                nc.tensor.matmul(
                                    q_ps,
                                    lhsT=wq_sb[:, 2 * ip:2 * ip + 2,
                                               j * 128:(j + 1) * 128],
                                    rhs=hfTs[t][:, 2 * ip:2 * ip + 2, :],
                                    perf_mode=DR,
                                    start=(ip == 0), stop=(ip == CI // 2 - 1))
                            nc.scalar.activation(
                                out=qTs[t][:, j, :],
                                in_=q_ps, func=AF.Identity,
                                bias=bq_sb[:, j:j + 1])

                    # prebuild query-tile-0 score pairs for this tile's kc
                    # range (kT tile t and qT tile 0 are now valid); per-tile
                    # emission spreads the ACT exp load evenly.
                    for p in (2 * t, 2 * t + 1):
                        pt0_cache.append(make_pair(0, p, ptc))

        # ================= Stage C: attention ============================
        # oT planes 0-3 in a 4-bank tile; per-query sums in a separate
        # 1-bank tile so its drain (srow) is independent of the casts.
        with tc.tile_pool(name="psO", bufs=1, space="PSUM") as psO, \
                tc.tile_pool(name="psU", bufs=1, space="PSUM") as psU:

            # epilogue part B: sums transpose + recip + y projection; the
            # caller places this where the PE has other queued work.
            def epilogue_b(qt, xr_t, srow, oT8a, oT8d):
                # bf16 PSUM writes need 4-byte alignment -> stride-2 columns
                st4 = psS.tile([128, 4, 2], BF16, tag="st",
                               name=f"st4_{qt}")
                for i in range(4):
                    nc.tensor.transpose(st4[:, i, 0:1],
                                        srow[0:1, i * 128:(i + 1) * 128],
                                        ident[0:1, 0:1])
                recip = stat.tile([128, 4], F32, tag="recip",
                                  name=f"recip_{qt}")
                nc.vector.reciprocal(out=recip, in_=st4[:, :, 0])
                for qc in range(4):
                    y_ps = psS.tile([128, C], F32, tag="st",
                                    name=f"y_ps_{qt}_{qc}")
                    nc.tensor.matmul(
                        y_ps,
                        lhsT=oT8a[:, :, qc * 128:(qc + 1) * 128],
                        rhs=wvp_sb[:, 0:2, :],
                        perf_mode=DR, start=True, stop=False)
                    nc.tensor.matmul(
                        y_ps,
                        lhsT=oT8d[:, :, qc * 128:(qc + 1) * 128],
                        rhs=wvp_sb[:, 2:4, :],
                        perf_mode=DR, start=False, stop=True)
                    y_sb = epi.tile([128, C], F32, tag="ysb", bufs=4)
                    nc.vector.scalar_tensor_tensor(
                        out=y_sb, in0=y_ps, scalar=recip[:, qc:qc + 1],
                        in1=xr_t[:, qc, :], op0=ALU.mult, op1=ALU.add)
                    nc.sync.dma_start(out=y_re[:, qt, qc, :], in_=y_sb)

            def emit_pv(p, pt2, oT_ps, sum_ps):
                tp_, l = (2 * p) // 4, (2 * p) % 4
                for cj in range(4):
                    nc.tensor.matmul(
                        oT_ps[:, cj, :],
                        lhsT=hNs[tp_][:, l:l + 2,
                                      cj * 128:(cj + 1) * 128],
                        rhs=pt2[:],
                        perf_mode=DR,
                        start=(p == 0), stop=(p == kp_n - 1))
                nc.tensor.matmul(
                    sum_ps,
                    lhsT=ones8,
                    rhs=pt2[:],
                    perf_mode=DR,
                    start=(p == 0), stop=(p == kp_n - 1))

            def drain_prev(qt, pending):
                """Split O^T drain: planes 01 in one ACT op -> oT8a,
                planes 23 in one DVE op -> oT8d (separate tiles, so the
                two engines genuinely run concurrently)."""
                pqt, poT, psum_t, pxr = pending
                oT8a = epi.tile([128, 2, C], F8, tag="ot8a",
                                name=f"ot8a_{pqt}")
                oT8d = epi.tile([128, 2, C], F8, tag="ot8d",
                                name=f"ot8d_{pqt}")
                # DVE cast emitted FIRST: cross-engine accesses to one tile
                # are ordered by emission, so this lets both casts overlap
                nc.vector.tensor_copy(out=oT8d, in_=poT[:, 2:4, :])
                nc.scalar.copy(out=oT8a, in_=poT[:, 0:2, :])
                srow = epi.tile([128, 512], BF16, tag="srow",
                                name=f"srow_{pqt}")
                # 4x so recip = 1/(4*sums) matches y_ps = o8 @ (4*Wp)
                nc.scalar.mul(srow, psum_t[:], WPSCALE)
                return (pqt, pxr, srow, oT8a, oT8d)

            pending = None
            for qt in range(qt_tiles):
                xr_t = work.tile([128, 4, C], F32, tag="xr")
                nc.sync.dma_start(out=xr_t, in_=xr_re[:, qt, :, :])
                prev_epi = None
                if qt == 0:
                    pts = dict(enumerate(pt0_cache))
                else:
                    # drain previous PSUM concurrently with five prebuilt
                    # score pairs (PE runway while casts + exps complete)
                    oT8s = drain_prev(qt, pending)
                    pts = {0: make_pair(qt, 0, ptp)}
                    pts[1] = make_pair(qt, 1, ptp)
                    pts[2] = make_pair(qt, 2, ptp)
                    pts[3] = make_pair(qt, 3, ptp)
                    pts[4] = make_pair(qt, 4, ptp)
                    prev_epi = oT8s
                    pending = None
                # planes 0-3: O^T[cj]; sums tile: per-query sums (replicated)
                oT_ps = psO.tile([128, 4, C], F32, tag="o", name=f"o_{qt}")
                sum_ps = psU.tile([128, C], F32, tag="u", name=f"u_{qt}")
                for p in range(kp_n):
                    if p + 3 < kp_n and (p + 3) not in pts:
                        pts[p + 3] = make_pair(qt, p + 3, ptp)
                    emit_pv(p, pts.pop(p), oT_ps, sum_ps)
                    if p == 0 and prev_epi is not None:
                        # previous tile's y projection fills the window
                        # between PV p0 and the next score chains
                        epilogue_b(*prev_epi)
                        prev_epi = None
                pending = (qt, oT_ps, sum_ps, xr_t)
            # tail: split casts (ACT || DVE) then chase the y projection
            oT8s = drain_prev(qt_tiles, pending)
            epilogue_b(*oT8s)

    nc.compile()
    return nc


def prep_host_inputs(x, ln_g, ln_b, Wq, bq, Wk, bk, Wv, bv, Wp, bp,
                     n_tok=N_TOK, nq=NQ, ncores=NCORES, nbatch=B):
    """Fold LN affine + linear biases on the host; build per-core maps."""
    f32 = np.float32
    x = np.asarray(x, f32)
    g = np.asarray(ln_g, f32)
    b = np.asarray(ln_b, f32)
    Wq = np.asarray(Wq, f32); Wk = np.asarray(Wk, f32)
    Wv = np.asarray(Wv, f32); Wp = np.asarray(Wp, f32)
    bq = np.asarray(bq, f32); bk = np.asarray(bk, f32)
    bv = np.asarray(bv, f32); bp = np.asarray(bp, f32)

    wq_e = g[:, None] * Wq
    bq_e = b @ Wq + bq
    wk_e = g[:, None] * Wk
    bk_e = b @ Wk + bk
    wv_e = g[:, None] * Wv
    bv_e = b @ Wv + bv
    resid_const = bv_e @ Wp + bp    # [C]

    ci = C // 128
    bq_pp = np.ascontiguousarray(bq_e.reshape(ci, 128).T).astype(f32)
    bk_pp = np.ascontiguousarray(bk_e.reshape(ci, 128).T).astype(f32)
    ident = np.eye(128, dtype=ml_dtypes.bfloat16)
    onesv = np.ones((128, 2, 128), dtype=NPF8)

    wvp = wv_e @ Wp
    shared = dict(
        wq=wq_e.astype(NPF8), wk=wk_e.astype(NPF8),
        wvp=(wvp * WPSCALE).astype(NPF8),
        bq=bq_pp, bk=bk_pp, ident=ident, ident8=np.eye(128, dtype=NPF8), ones=onesv,
    )

    xf = x.reshape(-1, C)  # flattened tokens, nbatch * n_tok rows
    halves = ncores // nbatch
    in_maps = []
    for core in range(ncores):
        bidx, half = divmod(core, halves)
        xb = xf[bidx * n_tok:(bidx + 1) * n_tok]
        if half:
            xp = np.ascontiguousarray(
                np.concatenate([xb[half * nq:], xb[:half * nq]], axis=0))
        else:
            xp = xb
        xr = (xp[:nq] + resid_const).astype(f32)
        m = dict(shared)
        m["x"] = np.ascontiguousarray(xp)
        m["xr"] = np.ascontiguousarray(xr)
        in_maps.append(m)
    return in_maps


_PROG = None


def _get_prog():
    global _PROG
    if _PROG is None:
        _PROG = build_program()
    return _PROG


def kernel(x, ln_g, ln_b, Wq, bq, Wk, bk, Wv, bv, Wp, bp, _trace=False,
           _tmpdir=None):
    global LAST_EXEC_NS, LAST_RESULT
    nc = _get_prog()
    in_maps = prep_host_inputs(x, ln_g, ln_b, Wq, bq, Wk, bk, Wv, bv, Wp, bp)
    res = run_bass_kernel_spmd(nc, in_maps, list(range(NCORES)), trace=_trace,
                               tmpdir=_tmpdir)
    LAST_EXEC_NS = res.exec_time_ns
    LAST_RESULT = res
    y = np.empty((B, N_TOK, C), np.float32)
    halves = NCORES // B
    for core in range(NCORES):
        bidx, half = divmod(core, halves)
        y[bidx, half * NQ:(half + 1) * NQ] = res.results[core]["y"]
    return y.reshape(B, Hh, Ww, C)


# revision 28
# speedup vs baseline: 1.0140x; 1.0140x over previous
"""Trainium2 Bass kernel for nn_AttentionBlock (B=4, H=W=64, C=512).

Strategy (8 cores, no collectives):
  - 2 cores per batch image; each core handles 2048 of the 4096 queries.
  - Key/token order is permuted per core so that each core's OWN query rows
    are tokens 0..2047 of its private x copy (softmax is invariant to key
    permutation as long as K and V use the same order).
  - All GEMMs run in fp8e4 with MatmulPerfMode.DoubleRow: lhsT/rhs carry
    [128, 2, *] channel- or key-chunk pairs so each matmul contracts 256
    elements at 1 column/cycle (the 157 TF/s fp8 peak).
  - Per core: LayerNorm (bn_stats on DVE; rstd via one optimized
    fast-inverse-sqrt Newton step), transpose hn to channel-major hfT
    (fp8 PE transpose), Q^T/K^T (channel-major) projections in fp8:
        S^T[k,q] = K^T.T @ Q^T     (PSUM fp32)
        P^T = exp(S^T/sqrt(C)-2.9) (ACT, scale+shift folded into the table)
        O^T[c,q] += V.T-pair @ P^T (PSUM planes 0-3, no output transpose)
        sums[q]  += ones.T @ P^T   (separate 1-bank PSUM tile)
        y = (O^T fp8) proj via 4*Wp back to [q,c]; y *= 1/(4*sums);
        out = y + x + const-biases
  - KEY SCHEDULING FACT: writes to one SBUF tile from different engines
    serialize in emission order (the tile tracker is cross-engine
    order-preserving per tile). All per-token-tile arrays (hn, hfT, K^T,
    Q^T) are therefore allocated as PER-TILE tiles with a SINGLE writer
    engine each, alternating engines across tiles, so ACT/DVE/GpSimd
    drain independent tiles concurrently. The O^T fp8 drain is likewise
    split into two tiles (planes 01 -> ACT, planes 23 -> DVE).
  - Tile boundaries: FIVE score pairs are prebuilt so the PE has a ~4.3us
    runway while the previous tile's PSUM is drained; the previous y
    projection/epilogue is emitted right after the first PV.
  - LN gamma/beta are folded into the QKV weights/biases on the host;
    bv/bp biases are folded into the residual input xr on the host; the
    softmax 1/sqrt(C) scale is applied by the ACT exp instruction.
"""

import os
import sys

import numpy as np
import ml_dtypes

try:
    import concourse.bass as bass
except ImportError:  # pragma: no cover - fresh-dir fallback
    for _p in ("/opt/trn_rl_repo", "/root/.axon_site/_ro/trn_rl_repo"):
        if os.path.isdir(_p) and _p not in sys.path:
            sys.path.insert(0, _p)
    import concourse.bass as bass

import concourse.bacc as bacc
import concourse.tile as tile
from concourse import mybir
from concourse.bass_utils import run_bass_kernel_spmd

F32 = mybir.dt.float32
BF16 = mybir.dt.bfloat16
F8 = mybir.dt.float8e4
AF = mybir.ActivationFunctionType
ALU = mybir.AluOpType
DR = mybir.MatmulPerfMode.DoubleRow
NPF8 = ml_dtypes.float8_e4m3

B, Hh, Ww, C = 4, 64, 64, 512
N_TOK = Hh * Ww          # 4096 tokens per image
NCORES = 8
NQ = N_TOK * B // NCORES  # 2048 queries per core
LN_EPS = 1e-3
CI = C // 128             # 4 channel chunks
SSCALE = 1.0 / float(np.sqrt(np.float32(C)))  # softmax scale, applied in exp
# exp(S*scale + ESHIFT): keeps P <= ~30 and O^T <= ~150 (fp8e4 max 240),
# so the O^T PSUM->SBUF copy is a pure cast. The extra ln(4) is undone by
# 4*Wp and the 4x srow scale (normalization is scale-invariant).
ESHIFT = -(1.5 + float(np.log(4.0)))
WPSCALE = 4.0

LAST_EXEC_NS = None
LAST_RESULT = None


def build_program(n_tok=N_TOK, nq=NQ):
    """Build the per-core Bass program (identical across cores)."""
    assert n_tok % 1024 == 0 and nq % 512 == 0
    nt_tiles = n_tok // 512   # n-tiles for K/V over all tokens
    qt_tiles = nq // 512      # q-tiles for this core's queries
    kc_n = n_tok // 128       # key chunks
    kp_n = kc_n // 2          # key chunk pairs

    nc = bacc.Bacc()
    if os.environ.get("BASS_CACHE_BUST"):
        nc.dram_tensor(f"cachebust_{os.environ['BASS_CACHE_BUST']}", [1, 1], F32)
    x_d = nc.dram_tensor("x", [n_tok, C], F32, kind="ExternalInput")
    xr_d = nc.dram_tensor("xr", [nq, C], F32, kind="ExternalInput")
    wq_d = nc.dram_tensor("wq", [C, C], F8, kind="ExternalInput")
    wk_d = nc.dram_tensor("wk", [C, C], F8, kind="ExternalInput")
    wvp_d = nc.dram_tensor("wvp", [C, C], F8, kind="ExternalInput")
    bq_d = nc.dram_tensor("bq", [128, CI], F32, kind="ExternalInput")
    bk_d = nc.dram_tensor("bk", [128, CI], F32, kind="ExternalInput")
    id_d = nc.dram_tensor("ident", [128, 128], BF16, kind="ExternalInput")
    id8_d = nc.dram_tensor("ident8", [128, 128], F8, kind="ExternalInput")
    on_d = nc.dram_tensor("ones", [128, 2, 128], F8, kind="ExternalInput")
    y_d = nc.dram_tensor("y", [nq, C], F32, kind="ExternalOutput")

    # token index mapping: tok = tile*512 + k*128 + p  (p = partition)
    x_re = x_d[:].rearrange("(t k p) c -> p t k c", p=128, k=4)
    xr_re = xr_d[:].rearrange("(t k p) c -> p t k c", p=128, k=4)
    y_re = y_d[:].rearrange("(t k p) c -> p t k c", p=128, k=4)

    from contextlib import ExitStack

    with ExitStack() as ctx:
        tc = ctx.enter_context(tile.TileContext(nc))
        consts = ctx.enter_context(tc.tile_pool(name="consts", bufs=1))
        big = ctx.enter_context(tc.tile_pool(name="big", bufs=1))
        work = ctx.enter_context(tc.tile_pool(name="work", bufs=3))
        stat = ctx.enter_context(tc.tile_pool(name="stat", bufs=4))
        ptp = ctx.enter_context(tc.tile_pool(name="ptp", bufs=6))
        ptc = ctx.enter_context(tc.tile_pool(name="ptc", bufs=kp_n))
        epi = ctx.enter_context(tc.tile_pool(name="epi", bufs=3))
        psS = ctx.enter_context(tc.tile_pool(name="psS", bufs=3, space="PSUM"))

        # ---- pipeline head: first x chunk, transpose identity, rest of
        # ---- x tiles 0/1 (chunk-grained so stats start early), weights.
        x_t0 = work.tile([128, 4, C], F32, tag="x", bufs=6)
        nc.sync.dma_start(out=x_t0[:, 0, :], in_=x_re[:, 0, 0, :])
        ident8 = consts.tile([128, 128], F8)
        nc.sync.dma_start(out=ident8, in_=id8_d[:])
        for k in range(1, 4):
            nc.sync.dma_start(out=x_t0[:, k, :], in_=x_re[:, 0, k, :])
        x_t1 = work.tile([128, 4, C], F32, tag="x", bufs=6, name="x_1")
        nc.sync.dma_start(out=x_t1[:, 0:2, :], in_=x_re[:, 1, 0:2, :])
        nc.sync.dma_start(out=x_t1[:, 2:4, :], in_=x_re[:, 1, 2:4, :])
        wk_sb = consts.tile([128, CI, C], F8)
        nc.sync.dma_start(out=wk_sb, in_=wk_d[:].rearrange("(ci p) co -> p ci co", p=128))
        wq_sb = consts.tile([128, CI, C], F8)
        nc.sync.dma_start(out=wq_sb, in_=wq_d[:].rearrange("(ci p) co -> p ci co", p=128))
        bk_sb = consts.tile([128, CI], F32)
        nc.sync.dma_start(out=bk_sb, in_=bk_d[:])
        bq_sb = consts.tile([128, CI], F32)
        nc.sync.dma_start(out=bq_sb, in_=bq_d[:])

        # stage-C constants: tiles now, DMAs issued after the first x
        # prefetches (they are not needed for tens of microseconds)
        wvp_sb = consts.tile([128, CI, C], F8)
        ones8 = consts.tile([128, 2, 128], F8)
        ident = consts.tile([128, 128], BF16)
        shf_sb = consts.tile([128, 1], F32)
        nc.vector.memset(shf_sb, ESHIFT)

        # ---- persistent activations: PER-TILE tiles, single writer each
        hfTs = [big.tile([128, CI, 512], F8, tag=f"hfT{t}", name=f"hfT{t}")
                for t in range(nt_tiles)]
        kTs = [big.tile([128, CI, 512], F8, tag=f"kT{t}", name=f"kT{t}")
               for t in range(nt_tiles)]
        hNs = [big.tile([128, 4, C], F8, tag=f"hN{t}", name=f"hN{t}")
               for t in range(nt_tiles)]
        qTs = [big.tile([128, CI, 512], F8, tag=f"qT{t}", name=f"qT{t}")
               for t in range(qt_tiles)]

        # scores + exp for one key chunk; pipelined ahead of PV use
        def st_exp(qt, kc, pt2, plane):
            kt_t, kcl = kTs[kc // 4], kc % 4
            s_ps = psS.tile([128, 512], F32, tag="st",
                            name=f"s_ps_{qt}_{kc}")
            for ip in range(CI // 2):
                nc.tensor.matmul(
                    s_ps,
                    lhsT=kt_t[:, 2 * ip:2 * ip + 2,
                              kcl * 128:(kcl + 1) * 128],
                    rhs=qTs[qt][:, 2 * ip:2 * ip + 2, :],
                    perf_mode=DR,
                    start=(ip == 0), stop=(ip == CI // 2 - 1))
            nc.scalar.activation(out=pt2[:, plane, :], in_=s_ps,
                                 func=AF.Exp, scale=SSCALE,
                                 bias=shf_sb)

        def make_pair(qt, p, pool):
            pt2 = pool.tile([128, 2, 512], F8, tag="pt",
                            name=f"pt_{qt}_{p}")
            st_exp(qt, 2 * p, pt2, 0)
            st_exp(qt, 2 * p + 1, pt2, 1)
            return pt2

        # ========= Stage A+B: LN, transpose, projections; the scores+exp
        # ========= for query tile 0 are interleaved as kT chunks land.
        pt0_cache = []
        xtiles = {0: x_t0, 1: x_t1}

        def fetch_x(t):
            if t not in xtiles and t < nt_tiles:
                xt = work.tile([128, 4, C], F32, tag="x", bufs=6,
                               name=f"x_{t}")
                nc.sync.dma_start(out=xt, in_=x_re[:, t, :, :])
                xtiles[t] = xt

        def emit_stats_n(chunks, label):
            """bn_stats (DVE) + one optimized fast-inverse-sqrt step.

            Kadlec's RcpSqrt1 (magic 0x5F1FFFF9, y*(1.68191409 -
            0.703952253*v*y^2)): max rel err 6.5e-4, well inside the fp8
            noise floor, and only 7 serial DVE micro-ops per batch.
            """
            m = len(chunks)
            mv8 = stat.tile([128, m, 2], F32, tag="mv", name=f"mv_{label}")
            for i, (xt, k) in enumerate(chunks):
                stats = stat.tile([128, 6], F32, tag="bnst")
                nc.vector.bn_stats(out=stats, in_=xt[:, k, :])
                nc.vector.bn_aggr(out=mv8[:, i, :], in_=stats)
            I32 = mybir.dt.int32
            veps = stat.tile([128, m], F32, tag="veps")
            nc.vector.tensor_scalar_add(out=veps, in0=mv8[:, :, 1],
                                        scalar1=LN_EPS)
            yb = stat.tile([128, m], I32, tag="yb")
            nc.vector.tensor_scalar(out=yb, in0=veps[:].bitcast(I32),
                                    scalar1=1, scalar2=None,
                                    op0=ALU.logical_shift_right)
            y0b = stat.tile([128, m], I32, tag="y0b")
            nc.vector.tensor_scalar(out=y0b, in0=yb, scalar1=0x5F1FFFF9,
                                    scalar2=-1,
                                    op0=ALU.subtract, op1=ALU.mult)
            t1 = stat.tile([128, m], F32, tag="nt1")
            nc.vector.tensor_tensor(out=t1, in0=y0b[:].bitcast(F32),
                                    in1=y0b[:].bitcast(F32), op=ALU.mult)
            t2 = stat.tile([128, m], F32, tag="nt2")
            nc.vector.tensor_tensor(out=t2, in0=t1, in1=veps, op=ALU.mult)
            t3 = stat.tile([128, m], F32, tag="nt3")
            nc.vector.tensor_scalar(out=t3, in0=t2, scalar1=-0.703952253,
                                    scalar2=1.68191409,
                                    op0=ALU.mult, op1=ALU.add)
            rstd8 = stat.tile([128, m], F32, tag="rstd", name=f"rstd_{label}")
            nc.vector.tensor_tensor(out=rstd8, in0=y0b[:].bitcast(F32),
                                    in1=t3, op=ALU.mult)
            # hn is applied as Identity(x*rstd + (-mu*rstd))
            b8 = stat.tile([128, m], F32, tag="b8", name=f"b8_{label}")
            nc.vector.scalar_tensor_tensor(out=b8, in0=mv8[:, :, 0],
                                           scalar=-1.0, in1=rstd8,
                                           op0=ALU.mult, op1=ALU.mult)
            return rstd8, b8

        ln_aff = {}
        with tc.tile_pool(name="psAB", bufs=5, space="PSUM") as psAB:
            for tp in range(nt_tiles // 2):
                # prefetch x deeply so tile boundaries never wait on HBM
                for ft in range(2 * tp + 2, min(2 * tp + 6, nt_tiles)):
                    fetch_x(ft)
                if tp == 0:
                    # stage-C constants ride behind the first prefetches
                    nc.sync.dma_start(out=wvp_sb, in_=wvp_d[:].rearrange(
                        "(ci p) co -> p ci co", p=128))
                    nc.sync.dma_start(out=ones8, in_=on_d[:])
                    nc.sync.dma_start(out=ident, in_=id_d[:])
                    # chunk 0 alone: its stats depend only on the first
                    # 256KB of x, so the first transpose starts early
                    t0_aff = []
                    for k in range(4):
                        rk, ck = emit_stats_n([(xtiles[0], k)], f"c{k}")
                        t0_aff.append((rk, ck, 0))
                    # tile 1 in two 2-chunk batches (chunk DMAs land in order)
                    r1a, c1a = emit_stats_n([(xtiles[1], 0), (xtiles[1], 1)],
                                            "t1a")
                    r1b, c1b = emit_stats_n([(xtiles[1], 2), (xtiles[1], 3)],
                                            "t1b")
                    ln_aff[0] = [t0_aff,
                                 [(r1a, c1a, 0), (r1a, c1a, 1),
                                  (r1b, c1b, 0), (r1b, c1b, 1)]]
                aff = ln_aff.pop(tp)
                for ti in range(2):
                    t = 2 * tp + ti
                    x_t = xtiles[t]
                    # next pair's stats: one 4-chunk batch per half-tile so
                    # the first tile's rstd lands well before the boundary
                    # (at tp==0 both batches wait until ti==1: tile 2's x
                    # DMA is still behind the startup head, and a blocked
                    # bn_stats would stall the DVE's small lookahead window)
                    if tp + 1 < nt_tiles // 2:
                        if ti == 0 and tp > 0:
                            ra, ca = emit_stats_n(
                                [(xtiles[2 * tp + 2], k) for k in range(4)],
                                f"pa{tp + 1}")
                            ln_aff.setdefault(tp + 1, [None, None])[0] = \
                                [(ra, ca, k) for k in range(4)]
                        elif ti == 1:
                            if tp == 0:
                                ra, ca = emit_stats_n(
                                    [(xtiles[2], k) for k in range(4)], "pa1")
                                ln_aff.setdefault(1, [None, None])[0] = \
                                    [(ra, ca, k) for k in range(4)]
                            rb, cb = emit_stats_n(
                                [(xtiles[2 * tp + 3], k) for k in range(4)],
                                f"pb{tp + 1}")
                            ln_aff[tp + 1][1] = [(rb, cb, k)
                                                 for k in range(4)]
                    # single-writer-per-tile engine assignment:
                    #   LN: ACT for tiles 0-1 (startup latency), then GpSimd
                    #   transpose copies: always DVE
                    #   K drains: always ACT; Q drains: DVE even t, ACT odd t
                    # only tile 0's LN on ACT (shortest cold-start path);
                    # tile 1's would queue behind tile 0's ACT drains
                    ln_eng = nc.scalar if t < 1 else nc.gpsimd
                    for k in range(4):
                        rstd8, b8, idx = aff[ti][k]
                        if ln_eng is nc.scalar:
                            nc.scalar.activation(out=hNs[t][:, k, :],
                                                 in_=x_t[:, k, :],
                                                 func=AF.Identity,
                                                 scale=rstd8[:, idx:idx + 1],
                                                 bias=b8[:, idx:idx + 1])
                        else:
                            nc.gpsimd.tensor_scalar(
                                out=hNs[t][:, k, :], in0=x_t[:, k, :],
                                scalar1=rstd8[:, idx:idx + 1],
                                scalar2=b8[:, idx:idx + 1],
                                op0=ALU.mult, op1=ALU.add)
                        # fp8 transpose must write PSUM at element step 2
                        tr_ps = psAB.tile([128, CI, 128, 2], F8, tag="ps")
                        for j in range(CI):
                            nc.tensor.transpose(
                                tr_ps[:, j, :, 0],
                                hNs[t][:, k, j * 128:(j + 1) * 128],
                                ident8)
                        nc.vector.tensor_copy(
                            out=hfTs[t][:, :, k * 128:(k + 1) * 128],
                            in_=tr_ps[:, :, :, 0])

                    # K^T columns for this tile (all drains on ACT: kTs[t]
                    # has a single writer engine)
                    for j in range(CI):
                        k_ps = psAB.tile([128, 512], F32, tag="ps")
                        for ip in range(CI // 2):
                            nc.tensor.matmul(
                                k_ps,
                                lhsT=wk_sb[:, 2 * ip:2 * ip + 2,
                                           j * 128:(j + 1) * 128],
                                rhs=hfTs[t][:, 2 * ip:2 * ip + 2, :],
                                perf_mode=DR,
                                start=(ip == 0), stop=(ip == CI // 2 - 1))
                        nc.scalar.activation(
                            out=kTs[t][:, j, :],
                            in_=k_ps, func=AF.Identity,
                            bias=bk_sb[:, j:j + 1])

                    # Q^T columns (only for this core's query range);
                    # drains on DVE for even t, ACT for odd t
                    if t < qt_tiles:
                        for j in range(CI):
                            q_ps = psAB.tile([128, 512], F32, tag="ps")
                            for ip in range(CI // 2):
                                nc.tensor.matmul(
                                    q_ps,
                                    lhsT=wq_sb[:, 2 * ip:2 * ip + 2,
                                               j * 128:(j + 1) * 128],
                                    rhs=hfTs[t][:, 2 * ip:2 * ip + 2, :],
                                    perf_mode=DR,
                                    start=(ip == 0), stop=(ip == CI // 2 - 1))
                            nc.scalar.activation(
                                out=qTs[t][:, j, :],
                                in_=q_ps, func=AF.Identity,
                                bias=bq_sb[:, j:j + 1])

                    # prebuild query-tile-0 score pairs for this tile's kc
                    # range (kT tile t and qT tile 0 are now valid); per-tile
                    # emission spreads the ACT exp load evenly.
                    for p in (2 * t, 2 * t + 1):
                        pt0_cache.append(make_pair(0, p, ptc))

        # ================= Stage C: attention ============================
        # oT planes 0-3 in a 4-bank tile; per-query sums in a separate
        # 1-bank tile so its drain (srow) is independent of the casts.
        with tc.tile_pool(name="psO", bufs=1, space="PSUM") as psO, \
                tc.tile_pool(name="psU", bufs=1, space="PSUM") as psU:

            # epilogue part B: sums transpose + recip + y projection; the
            # caller places this where the PE has other queued work.
            def epilogue_b(qt, xr_t, srow, oT8a, oT8d):
                # bf16 PSUM writes need 4-byte alignment -> stride-2 columns
                st4 = psS.tile([128, 4, 2], BF16, tag="st",
                               name=f"st4_{qt}")
                for i in range(4):
                    nc.tensor.transpose(st4[:, i, 0:1],
                                        srow[0:1, i * 128:(i + 1) * 128],
                                        ident[0:1, 0:1])
                recip = stat.tile([128, 4], F32, tag="recip",
                                  name=f"recip_{qt}")
                nc.vector.reciprocal(out=recip, in_=st4[:, :, 0])
                for qc in range(4):
                    y_ps = psS.tile([128, C], F32, tag="st",
                                    name=f"y_ps_{qt}_{qc}")
                    nc.tensor.matmul(
                        y_ps,
                        lhsT=oT8a[:, :, qc * 128:(qc + 1) * 128],
                        rhs=wvp_sb[:, 0:2, :],
                        perf_mode=DR, start=True, stop=False)
                    nc.tensor.matmul(
                        y_ps,
                        lhsT=oT8d[:, :, qc * 128:(qc + 1) * 128],
                        rhs=wvp_sb[:, 2:4, :],
                        perf_mode=DR, start=False, stop=True)
                    y_sb = epi.tile([128, C], F32, tag="ysb", bufs=4)
                    nc.vector.scalar_tensor_tensor(
                        out=y_sb, in0=y_ps, scalar=recip[:, qc:qc + 1],
                        in1=xr_t[:, qc, :], op0=ALU.mult, op1=ALU.add)
                    nc.sync.dma_start(out=y_re[:, qt, qc, :], in_=y_sb)

            def emit_pv(p, pt2, oT_ps, sum_ps):
                tp_, l = (2 * p) // 4, (2 * p) % 4
                for cj in range(4):
                    nc.tensor.matmul(
                        oT_ps[:, cj, :],
                        lhsT=hNs[tp_][:, l:l + 2,
                                      cj * 128:(cj + 1) * 128],
                        rhs=pt2[:],
                        perf_mode=DR,
                        start=(p == 0), stop=(p == kp_n - 1))
                nc.tensor.matmul(
                    sum_ps,
                    lhsT=ones8,
                    rhs=pt2[:],
                    perf_mode=DR,
                    start=(p == 0), stop=(p == kp_n - 1))

            def drain_prev(qt, pending):
                """Split O^T drain: planes 01 in one ACT op -> oT8a,
                planes 23 in one DVE op -> oT8d (separate tiles, so the
                two engines genuinely run concurrently)."""
                pqt, poT, psum_t, pxr = pending
                oT8a = epi.tile([128, 2, C], F8, tag="ot8a",
                                name=f"ot8a_{pqt}")
                oT8d = epi.tile([128, 2, C], F8, tag="ot8d",
                                name=f"ot8d_{pqt}")
                # DVE cast emitted FIRST: cross-engine accesses to one tile
                # are ordered by emission, so this lets both casts overlap
                nc.vector.tensor_copy(out=oT8d, in_=poT[:, 2:4, :])
                nc.scalar.copy(out=oT8a, in_=poT[:, 0:2, :])
                srow = epi.tile([128, 512], BF16, tag="srow",
                                name=f"srow_{pqt}")
                # 4x so recip = 1/(4*sums) matches y_ps = o8 @ (4*Wp)
                nc.scalar.mul(srow, psum_t[:], WPSCALE)
                return (pqt, pxr, srow, oT8a, oT8d)

            pending = None
            for qt in range(qt_tiles):
                xr_t = work.tile([128, 4, C], F32, tag="xr")
                nc.sync.dma_start(out=xr_t, in_=xr_re[:, qt, :, :])
                prev_epi = None
                if qt == 0:
                    pts = dict(enumerate(pt0_cache))
                else:
                    # drain previous PSUM concurrently with five prebuilt
                    # score pairs (PE runway while casts + exps complete)
                    oT8s = drain_prev(qt, pending)
                    pts = {0: make_pair(qt, 0, ptp)}
                    pts[1] = make_pair(qt, 1, ptp)
                    pts[2] = make_pair(qt, 2, ptp)
                    pts[3] = make_pair(qt, 3, ptp)
                    pts[4] = make_pair(qt, 4, ptp)
                    pts[5] = make_pair(qt, 5, ptp)
                    prev_epi = oT8s
                    pending = None
                # planes 0-3: O^T[cj]; sums tile: per-query sums (replicated)
                oT_ps = psO.tile([128, 4, C], F32, tag="o", name=f"o_{qt}")
                sum_ps = psU.tile([128, C], F32, tag="u", name=f"u_{qt}")
                for p in range(kp_n):
                    if p + 3 < kp_n and (p + 3) not in pts:
                        pts[p + 3] = make_pair(qt, p + 3, ptp)
                    emit_pv(p, pts.pop(p), oT_ps, sum_ps)
                    if p == 1 and prev_epi is not None:
                        # previous tile's y projection fills the window
                        # between PV p0 and the next score chains
                        epilogue_b(*prev_epi)
                        prev_epi = None
                pending = (qt, oT_ps, sum_ps, xr_t)
            # tail: split casts (ACT || DVE) then chase the y projection
            oT8s = drain_prev(qt_tiles, pending)
            epilogue_b(*oT8s)

    nc.compile()
    return nc


def prep_host_inputs(x, ln_g, ln_b, Wq, bq, Wk, bk, Wv, bv, Wp, bp,
                     n_tok=N_TOK, nq=NQ, ncores=NCORES, nbatch=B):
    """Fold LN affine + linear biases on the host; build per-core maps."""
    f32 = np.float32
    x = np.asarray(x, f32)
    g = np.asarray(ln_g, f32)
    b = np.asarray(ln_b, f32)
    Wq = np.asarray(Wq, f32); Wk = np.asarray(Wk, f32)
    Wv = np.asarray(Wv, f32); Wp = np.asarray(Wp, f32)
    bq = np.asarray(bq, f32); bk = np.asarray(bk, f32)
    bv = np.asarray(bv, f32); bp = np.asarray(bp, f32)

    wq_e = g[:, None] * Wq
    bq_e = b @ Wq + bq
    wk_e = g[:, None] * Wk
    bk_e = b @ Wk + bk
    wv_e = g[:, None] * Wv
    bv_e = b @ Wv + bv
    resid_const = bv_e @ Wp + bp    # [C]

    ci = C // 128
    bq_pp = np.ascontiguousarray(bq_e.reshape(ci, 128).T).astype(f32)
    bk_pp = np.ascontiguousarray(bk_e.reshape(ci, 128).T).astype(f32)
    ident = np.eye(128, dtype=ml_dtypes.bfloat16)
    onesv = np.ones((128, 2, 128), dtype=NPF8)

    wvp = wv_e @ Wp
    shared = dict(
        wq=wq_e.astype(NPF8), wk=wk_e.astype(NPF8),
        wvp=(wvp * WPSCALE).astype(NPF8),
        bq=bq_pp, bk=bk_pp, ident=ident, ident8=np.eye(128, dtype=NPF8), ones=onesv,
    )

    xf = x.reshape(-1, C)  # flattened tokens, nbatch * n_tok rows
    halves = ncores // nbatch
    in_maps = []
    for core in range(ncores):
        bidx, half = divmod(core, halves)
        xb = xf[bidx * n_tok:(bidx + 1) * n_tok]
        if half:
            xp = np.ascontiguousarray(
                np.concatenate([xb[half * nq:], xb[:half * nq]], axis=0))
        else:
            xp = xb
        xr = (xp[:nq] + resid_const).astype(f32)
        m = dict(shared)
        m["x"] = np.ascontiguousarray(xp)
        m["xr"] = np.ascontiguousarray(xr)
        in_maps.append(m)
    return in_maps


_PROG = None


def _get_prog():
    global _PROG
    if _PROG is None:
        _PROG = build_program()
    return _PROG


def kernel(x, ln_g, ln_b, Wq, bq, Wk, bk, Wv, bv, Wp, bp, _trace=False,
           _tmpdir=None):
    global LAST_EXEC_NS, LAST_RESULT
    nc = _get_prog()
    in_maps = prep_host_inputs(x, ln_g, ln_b, Wq, bq, Wk, bk, Wv, bv, Wp, bp)
    res = run_bass_kernel_spmd(nc, in_maps, list(range(NCORES)), trace=_trace,
                               tmpdir=_tmpdir)
    LAST_EXEC_NS = res.exec_time_ns
    LAST_RESULT = res
    y = np.empty((B, N_TOK, C), np.float32)
    halves = NCORES // B
    for core in range(NCORES):
        bidx, half = divmod(core, halves)
        y[bidx, half * NQ:(half + 1) * NQ] = res.results[core]["y"]
    return y.reshape(B, Hh, Ww, C)
